# revision 1
# baseline (speedup 1.0000x reference)
"""Trainium2 Bass kernel for nn_CrossDomainFusion.

Data-parallel over batch: core b handles batch b (B=8 across 8 cores),
weights replicated. See build_nc() for the per-core program.

Math (per batch):
  time branch : ConvTranspose1d(k=4,s=2,p=1) + linear  ->  H_time [2048,512]
                (conv and projection weights are fused on the host into 4
                 [256,512] matrices, one per even/odd tap)
  spec branch : reshape + linear                       ->  H_spec [2048,512]
  S = H_time @ H_spec^T / sqrt(512)
  out = concat[ softmax_rows(S) @ H_spec, softmax_rows(S^T) @ H_time ]

Implementation notes:
  - All matmuls run in bf16 with fp32 PSUM accumulation.
  - Logits are O(1) for these inputs, so softmax skips the max-subtraction.
  - exp(S) is computed ONCE, as tiles e1 = [spec 128, time 512]; those are
    directly the PV lhsT for the time-query pass, and their 128x128 PE
    transposes are the PV lhsT pieces for the spec-query pass.  This avoids
    recomputing the logit matmuls for the second softmax orientation.
  - ACT's accum_out on the exp gives the spec-side softmax denominators for
    free; the time-side denominators come from strided free-axis reduces of
    the transposed tiles.
  - Normalization is deferred until both accumulations finish; output rows
    for the time branch are written even/odd interleaved via strided DMA.
"""

import sys

sys.path.insert(0, "/opt/trn_rl_repo")

from contextlib import ExitStack

import ml_dtypes
import numpy as np

import concourse.bacc as bacc
import concourse.tile as tile
from concourse import mybir
from concourse.bass_utils import run_bass_kernel_spmd
from concourse.masks import make_identity

BF16 = mybir.dt.bfloat16
F32 = mybir.dt.float32
NPBF16 = ml_dtypes.bfloat16

B, L, C, D, S, CF = 8, 1024, 256, 512, 2048, 192
SCALE = 1.0 / float(np.sqrt(D))
EXP = mybir.ActivationFunctionType.Exp
ADD = mybir.AluOpType.add
AXX = mybir.AxisListType.X


def build_nc():
    nc = bacc.Bacc("TRN2", target_bir_lowering=False, debug=False, num_devices=8)
    xt = nc.declare_dram_parameter("xt", [C, L + 2], BF16, isOutput=False)
    sfa = nc.declare_dram_parameter("sfa", [CF + 1, S], BF16, isOutput=False)
    wt = nc.declare_dram_parameter("wt", [4, C, D], BF16, isOutput=False)
    wsp = nc.declare_dram_parameter("wsp", [CF + 1, D], BF16, isOutput=False)
    bt32 = nc.declare_dram_parameter("bt32", [128, 4], F32, isOutput=False)
    out = nc.declare_dram_parameter("out", [S, 2 * D], F32, isOutput=True)
    out_r = out.ap().rearrange("(m two) h -> two m h", two=2)

    with ExitStack() as ctx:
        tc = ctx.enter_context(tile.TileContext(nc))
        const = ctx.enter_context(tc.tile_pool(name="const", bufs=1))
        hpool = ctx.enter_context(tc.tile_pool(name="h", bufs=1))
        upool = ctx.enter_context(tc.tile_pool(name="u", bufs=1))
        epool = ctx.enter_context(tc.tile_pool(name="e", bufs=6))
        spool = ctx.enter_context(tc.tile_pool(name="stage", bufs=3))
        ps = ctx.enter_context(tc.tile_pool(name="ps", bufs=2, space="PSUM"))
        pu = ctx.enter_context(tc.tile_pool(name="pu", bufs=4, space="PSUM"))

        # ---- input loads (XT + WT first: they gate the first matmuls) ----
        XT = []
        for c in range(2):
            t = const.tile([128, L + 2], BF16, name=f"xt{c}", tag=f"xt{c}")
            nc.sync.dma_start(t[:], xt[c * 128 : (c + 1) * 128, :])
            XT.append(t)
        WT = []
        for t_ in range(4):
            row = []
            for c in range(2):
                w = const.tile([128, D], BF16, name=f"wt{t_}{c}", tag=f"wt{t_}{c}")
                nc.sync.dma_start(w[:], wt[t_, c * 128 : (c + 1) * 128, :])
                row.append(w)
            WT.append(row)
        BT = const.tile([128, 4], F32, tag="bt")
        nc.sync.dma_start(BT[:], bt32[:])
        SFA0 = const.tile([128, S], BF16, tag="sfa0")
        nc.sync.dma_start(SFA0[:], sfa[0:128, :])
        SFA1 = const.tile([65, S], BF16, tag="sfa1")
        nc.sync.dma_start(SFA1[:], sfa[128:193, :])
        WSP0 = const.tile([128, D], BF16, tag="wsp0")
        nc.sync.dma_start(WSP0[:], wsp[0:128, :])
        WSP1 = const.tile([65, D], BF16, tag="wsp1")
        nc.sync.dma_start(WSP1[:], wsp[128:193, :])
        ID = const.tile([128, 128], BF16, tag="id")
        make_identity(nc, ID[:])
        warm = const.tile([128, 512], BF16, tag="warm")
        nc.gpsimd.memset(warm[:], 0.0)
        for _ in range(16):
            wp = ps.tile([128, 512], F32, name="wps", tag="ps")
            nc.tensor.matmul(wp[:], lhsT=ID[:], rhs=warm[:], start=True, stop=True)

        # ---- persistent SBUF tensors ----
        HtT = [hpool.tile([128, S], BF16, name=f"htt{d}", tag=f"htt{d}") for d in range(4)]
        HsT = [hpool.tile([128, S], BF16, name=f"hst{d}", tag=f"hst{d}") for d in range(4)]
        HtN = [hpool.tile([128, D], BF16, name=f"htn{k}", tag=f"htn{k}") for k in range(16)]
        HsN = [hpool.tile([128, D], BF16, name=f"hsn{k}", tag=f"hsn{k}") for k in range(16)]
        U2 = [upool.tile([128, D], F32, name=f"u2{k}", tag=f"u2{k}") for k in range(16)]
        # spec-side denominators: partition = spec, col = sc*4 + tsl
        DSraw = hpool.tile([128, 128], F32, tag="dsraw")
        # time-side denominators: partition = time, col = k*16 + sc
        DTraw = hpool.tile([128, 256], F32, tag="dtraw")
        DTr3 = DTraw.rearrange("p (k s) -> p k s", s=16)
        DS = hpool.tile([128, 16], F32, tag="ds")
        DT = hpool.tile([128, 16], F32, tag="dt")
        RS = hpool.tile([128, 16], F32, tag="rs")
        RT = hpool.tile([128, 16], F32, tag="rt")

        # ---- H phase: HtT[d][p, 0:1024]=even positions, [1024:2048]=odd ----
        # wt rows: 0=W1t(x[m],even) 1=W3t(x[m-1],even) 2=W2t(x[m],odd) 3=W0t(x[m+1],odd)
        # XT col m+1 <-> x[m]
        taps = [((0, 1), (1, 0)), ((2, 1), (3, 2))]  # (wt_idx, xt_offset)
        for d in range(4):
            for half in range(2):
                for ms in range(2):
                    hidx = d * 4 + half * 2 + ms
                    if hidx % 3 == 0:
                        p = ps.tile([128, 512], F32, name="hps", tag="ps")
                    else:
                        p = pu.tile([128, 512], F32, name="hps", tag="u")
                    n = 0
                    for ti, off in taps[half]:
                        for c in range(2):
                            nc.tensor.matmul(
                                p[:],
                                lhsT=WT[ti][c][:, d * 128 : (d + 1) * 128],
                                rhs=XT[c][:, off + ms * 512 : off + ms * 512 + 512],
                                start=(n == 0),
                                stop=(n == 3),
                            )
                            n += 1
                    col = half * 1024 + ms * 512
                    nc.vector.tensor_scalar_add(
                        HtT[d][:, col : col + 512], p[:], BT[:, d : d + 1]
                    )

        # HsT (bias folded: wsp row 192 = b_sproj, sfa row 192 = ones)
        for d in range(4):
            for ts_ in range(4):
                if (d * 4 + ts_) % 3 == 0:
                    p = ps.tile([128, 512], F32, name="hps", tag="ps")
                else:
                    p = pu.tile([128, 512], F32, name="hps", tag="u")
                nc.tensor.matmul(
                    p[:],
                    lhsT=WSP0[:, d * 128 : (d + 1) * 128],
                    rhs=SFA0[:, ts_ * 512 : (ts_ + 1) * 512],
                    start=True,
                    stop=False,
                )
                nc.tensor.matmul(
                    p[:],
                    lhsT=WSP1[:, d * 128 : (d + 1) * 128],
                    rhs=SFA1[:, ts_ * 512 : (ts_ + 1) * 512],
                    start=False,
                    stop=True,
                )
                if ts_ % 2 == 0:
                    nc.scalar.copy(HsT[d][:, ts_ * 512 : (ts_ + 1) * 512], p[:])
                else:
                    nc.vector.tensor_copy(HsT[d][:, ts_ * 512 : (ts_ + 1) * 512], p[:])

        # HsN natural [t, h] (bias folded the same way)
        for k in range(16):
            if k % 3 == 0:
                p = ps.tile([128, 512], F32, name="hps", tag="ps")
            else:
                p = pu.tile([128, 512], F32, name="hps", tag="u")
            nc.tensor.matmul(
                p[:],
                lhsT=SFA0[:, k * 128 : (k + 1) * 128],
                rhs=WSP0[:],
                start=True,
                stop=False,
            )
            nc.tensor.matmul(
                p[:],
                lhsT=SFA1[:, k * 128 : (k + 1) * 128],
                rhs=WSP1[:],
                start=False,
                stop=True,
            )
            if k % 2 == 0:
                nc.vector.tensor_copy(HsN[k][:], p[:])
            else:
                nc.scalar.copy(HsN[k][:], p[:])

        # HtN natural via PE transposes of HtT (bias already in), 4 packed per bank
        for k in range(16):
            if k % 2 == 0:
                tp = ps.tile([128, 512], BF16, name="htp", tag="tr", bufs=1)
            else:
                tp = pu.tile([128, 512], BF16, name="htp2", tag="u2p", bufs=1)
            for d in range(4):
                nc.tensor.transpose(
                    tp[:, d * 128 : (d + 1) * 128],
                    HtT[d][:, k * 128 : (k + 1) * 128],
                    ID[:],
                )
            if k % 2 == 0:
                nc.scalar.copy(HtN[k][:], tp[:])
            else:
                nc.vector.tensor_copy(HtN[k][:], tp[:])

        # ---- attention: exp(S) computed once ----
        # e1 tile [spec-chunk sc 128, time-slice tsl 512] = exp(scale*S)^T slice
        for tsl in range(4):
            u1 = [pu.tile([128, D], F32, name="u1psum", tag="u") for _ in range(4)]
            for sc in range(16):
                p = ps.tile([128, 512], F32, name="spsum", tag="ps")
                for d in range(4):
                    nc.tensor.matmul(
                        p[:],
                        lhsT=HsT[d][:, sc * 128 : (sc + 1) * 128],
                        rhs=HtT[d][:, tsl * 512 : (tsl + 1) * 512],
                        start=(d == 0),
                        stop=(d == 3),
                    )
                e1 = epool.tile([128, 512], BF16, name="e1", tag="e")
                for hf in range(2):
                    nc.scalar.activation(
                        e1[:, hf * 256 : (hf + 1) * 256],
                        p[:, hf * 256 : (hf + 1) * 256],
                        EXP,
                        scale=SCALE,
                        accum_out=DSraw[
                            :,
                            (sc * 4 + tsl) * 2 + hf : (sc * 4 + tsl) * 2 + hf + 1,
                        ],
                    )
                # PV1 (time queries): U1[time, h] accumulates over spec chunks
                for q in range(4):
                    nc.tensor.matmul(
                        u1[q][:],
                        lhsT=e1[:, q * 128 : (q + 1) * 128],
                        rhs=HsN[sc][:],
                        start=(sc == 0),
                        stop=(sc == 15),
                    )
                # transpose e1 -> 4 pieces [time-chunk tsl*4+q, spec sc]
                tp = ps.tile([128, 512], BF16, name="etp", tag="tr", bufs=1)
                for q in range(4):
                    nc.tensor.transpose(
                        tp[:, q * 128 : (q + 1) * 128],
                        e1[:, q * 128 : (q + 1) * 128],
                        ID[:],
                    )
                eTs = epool.tile([128, 512], BF16, name="eTs", tag="ets", bufs=4)
                nc.vector.tensor_copy(eTs[:], tp[:])
                # time-side denominator pieces: free-sum over spec within piece
                nc.vector.reduce_sum(
                    DTr3[:, tsl * 4 : tsl * 4 + 4, sc : sc + 1],
                    eTs.rearrange("p (q s) -> p q s", s=128),
                    axis=AXX,
                )
                # PV2 (spec queries): partial over this time-slice, SBUF-accumulated
                u2p = pu.tile([128, 512], F32, name="u2p", tag="u2p", bufs=1)
                for q in range(4):
                    nc.tensor.matmul(
                        u2p[:],
                        lhsT=eTs[:, q * 128 : (q + 1) * 128],
                        rhs=HtN[tsl * 4 + q][:],
                        start=(q == 0),
                        stop=(q == 3),
                    )
                if tsl == 0:
                    nc.vector.tensor_copy(U2[sc][:], u2p[:])
                elif tsl < 3:
                    nc.vector.tensor_tensor(U2[sc][:], U2[sc][:], u2p[:], op=ADD)
                else:
                    nc.vector.tensor_tensor(U2[sc][:], U2[sc][:], u2p[:], op=ADD)
                    nc.vector.reduce_sum(
                        DS[:, sc : sc + 1], DSraw[:, sc * 8 : (sc + 1) * 8], axis=AXX
                    )
                    nc.vector.reciprocal(RS[:, sc : sc + 1], DS[:, sc : sc + 1])
                    o2 = spool.tile([128, D], F32, name="o2", tag="o")
                    nc.vector.tensor_scalar_mul(o2[:], U2[sc][:], RS[:, sc : sc + 1])
                    nc.sync.dma_start(out[sc * 128 : (sc + 1) * 128, D : 2 * D], o2[:])
            for q in range(4):
                k = tsl * 4 + q
                nc.vector.reduce_sum(DT[:, k : k + 1], DTr3[:, k, :], axis=AXX)
                nc.vector.reciprocal(RT[:, k : k + 1], DT[:, k : k + 1])
                o = spool.tile([128, D], F32, name="o1", tag="o")
                nc.scalar.mul(o[:], u1[q][:], RT[:, k : k + 1])
                par, m0 = (0, k * 128) if k < 8 else (1, (k - 8) * 128)
                nc.sync.dma_start(out_r[par, m0 : m0 + 128, 0:D], o[:])

    nc.compile()
    return nc


def make_in_maps(
    time_features,
    spec_features,
    w_conv,
    b_conv,
    w_tproj,
    b_tproj,
    w_sproj,
    b_sproj,
):
    time_features = np.asarray(time_features, np.float32)
    spec_features = np.asarray(spec_features, np.float32)
    w_conv = np.asarray(w_conv, np.float32)
    b_conv = np.asarray(b_conv, np.float32)
    w_tproj = np.asarray(w_tproj, np.float32)
    b_tproj = np.asarray(b_tproj, np.float32)
    w_sproj = np.asarray(w_sproj, np.float32)
    b_sproj = np.asarray(b_sproj, np.float32)

    # fused conv+tproj weights, tap order [W1t, W3t, W2t, W0t]
    wk = [w_conv[:, :, k] @ w_tproj.T for k in range(4)]  # (in=256, 512)
    wt = np.stack([wk[1], wk[3], wk[2], wk[0]]).astype(NPBF16)
    wsp = np.concatenate([w_sproj.T, b_sproj[None, :]], 0).astype(NPBF16)
    bt = b_conv @ w_tproj.T + b_tproj
    bt32 = np.ascontiguousarray(bt.reshape(4, 128).T, dtype=np.float32)

    in_maps = []
    for b in range(B):
        xt = np.zeros((C, L + 2), NPBF16)
        xt[:, 1 : L + 1] = time_features[b].T.astype(NPBF16)
        sfa = np.concatenate(
            [spec_features[b].reshape(CF, S), np.ones((1, S), np.float32)], 0
        ).astype(NPBF16)
        in_maps.append(
            {"xt": xt, "sfa": sfa, "wt": wt, "wsp": wsp, "bt32": bt32}
        )
    return in_maps


_NC_CACHE = None


def get_nc():
    global _NC_CACHE
    if _NC_CACHE is None:
        _NC_CACHE = build_nc()
    return _NC_CACHE


def kernel(**inputs) -> np.ndarray:
    nc = get_nc()
    in_maps = make_in_maps(**inputs)
    res = run_bass_kernel_spmd(nc, in_maps, list(range(B)))
    return np.stack([res.results[i]["out"] for i in range(B)])


if __name__ == "__main__":
    rng = np.random.default_rng(0)
    ins = {
        "time_features": rng.standard_normal((B, L, C)).astype(np.float32),
        "spec_features": rng.standard_normal((B, 3, 64, S)).astype(np.float32),
        "w_conv": (rng.standard_normal((C, C, 4)) * 0.05).astype(np.float32),
        "b_conv": (rng.standard_normal(C) * 0.05).astype(np.float32),
        "w_tproj": (rng.standard_normal((D, C)) * 0.05).astype(np.float32),
        "b_tproj": (rng.standard_normal(D) * 0.05).astype(np.float32),
        "w_sproj": (rng.standard_normal((D, CF)) * 0.05).astype(np.float32),
        "b_sproj": (rng.standard_normal(D) * 0.05).astype(np.float32),
    }
    out = kernel(**ins)
    print("out", out.shape, out.dtype, float(np.abs(out).max()))



# revision 12
# speedup vs baseline: 1.5368x; 1.5368x over previous
"""Trainium2 Bass kernel for nn_CrossDomainFusion.

Data-parallel over batch: core b handles batch b (B=8 across 8 cores),
weights replicated.

Math (per batch), exploiting that both value matrices are low-rank:
  T  = conv_transpose(x)                 [2048, 256]   (pre-projection)
  Ht = T @ Wt' + bt                      (bt folded via logit bias + output row)
  Hs = P~^T @ Ws'                        P~ = spec features + ones row [193, 2048]
  S  = Hs @ Ht^T = P~^T @ (M @ T^T) + BL·1^T,  M = Ws' @ Wt  (host)  [193, 256]
  E  = exp(S/sqrt(512))  -> fp8, both orientations (PE transpose)
  fused_time = (E^T @ P~^T-fp8) @ Ws' / dt   (dt = col 192 of R, ones-row trick)
  fused_spec = (E @ [T,1]-fp8) @ Wt' / ds + bt-row

The two big attention-apply matmuls (R = E^T@P~^T, U = E@[T,1]) run as
fp8 DoubleRow matmuls (2 k-tiles of 128 per instruction); contraction
on the projection side is 193/257 instead of 512, roughly halving PE
work vs. the direct H-space formulation. Logit path stays bf16.
"""

import sys

sys.path.insert(0, "/opt/trn_rl_repo")

from contextlib import ExitStack

import ml_dtypes
import numpy as np

import concourse.bacc as bacc
import concourse.tile as tile
from concourse import mybir
from concourse.bass_utils import run_bass_kernel_spmd
from concourse.masks import make_identity

BF16 = mybir.dt.bfloat16
F8 = mybir.dt.float8e4
F32 = mybir.dt.float32
NPBF16 = ml_dtypes.bfloat16
NPF8 = ml_dtypes.float8_e4m3
DR = mybir.MatmulPerfMode.DoubleRow

B, L, C, D, S, CF = 8, 1024, 256, 512, 2048, 192
SCALE = 1.0 / float(np.sqrt(D))
EXP = mybir.ActivationFunctionType.Exp
ADD = mybir.AluOpType.add


def build_nc():
    nc = bacc.Bacc("TRN2", target_bir_lowering=False, debug=False, num_devices=8)
    xt = nc.declare_dram_parameter("xt", [C, L + 2], BF16, isOutput=False)
    wct = nc.declare_dram_parameter("wct", [4, C, C], BF16, isOutput=False)
    mT = nc.declare_dram_parameter("mT", [C, CF + 1], BF16, isOutput=False)
    g0c = nc.declare_dram_parameter("g0c", [CF + 1, 1], BF16, isOutput=False)
    sfa = nc.declare_dram_parameter("sfa", [CF + 1, S], BF16, isOutput=False)
    p8t = nc.declare_dram_parameter("p8t", [8, 128, 2 * (CF + 1)], F8, isOutput=False)
    wsp = nc.declare_dram_parameter("wsp", [CF + 1, D], BF16, isOutput=False)
    wtn = nc.declare_dram_parameter("wtn", [C, D], BF16, isOutput=False)
    btt = nc.declare_dram_parameter("btt", [128, D], F32, isOutput=False)
    out = nc.declare_dram_parameter("out", [S, 2 * D], F32, isOutput=True)
    out_r = out.ap().rearrange("(m two) h -> two m h", two=2)

    with ExitStack() as ctx:
        tc = ctx.enter_context(tile.TileContext(nc))
        const = ctx.enter_context(tc.tile_pool(name="const", bufs=1))
        hpool = ctx.enter_context(tc.tile_pool(name="h", bufs=1))
        epool = ctx.enter_context(tc.tile_pool(name="e", bufs=9))
        tpool = ctx.enter_context(tc.tile_pool(name="t", bufs=3))
        spool = ctx.enter_context(tc.tile_pool(name="stage", bufs=3))
        ps = ctx.enter_context(tc.tile_pool(name="ps", bufs=2, space="PSUM"))
        pe8 = ctx.enter_context(tc.tile_pool(name="pe8", bufs=1, space="PSUM"))
        pr = ctx.enter_context(tc.tile_pool(name="pr", bufs=1, space="PSUM"))
        pu = ctx.enter_context(tc.tile_pool(name="pu", bufs=1, space="PSUM"))
        ptr = ctx.enter_context(tc.tile_pool(name="ptr", bufs=1, space="PSUM"))
        po = ctx.enter_context(tc.tile_pool(name="po", bufs=1, space="PSUM"))

        # ---- input loads ----
        XT = []
        for c in range(2):
            t = const.tile([128, L + 2], BF16, name=f"xt{c}", tag=f"xt{c}")
            nc.sync.dma_start(t[:], xt[c * 128 : (c + 1) * 128, :])
            XT.append(t)
        WCT = []
        for t_ in range(4):
            row = []
            for c in range(2):
                w = const.tile([128, C], BF16, name=f"wct{t_}{c}", tag=f"wct{t_}{c}")
                nc.sync.dma_start(w[:], wct[t_, c * 128 : (c + 1) * 128, :])
                row.append(w)
            WCT.append(row)
        MT = []
        for c in range(2):
            w = const.tile([128, CF + 1], BF16, name=f"mt{c}", tag=f"mt{c}")
            nc.sync.dma_start(w[:], mT[c * 128 : (c + 1) * 128, :])
            MT.append(w)
        G0C0 = const.tile([128, 1], BF16, tag="g0c0")
        nc.sync.dma_start(G0C0[:], g0c[0:128, :])
        G0C1 = const.tile([65, 1], BF16, tag="g0c1")
        nc.sync.dma_start(G0C1[:], g0c[128:193, :])
        SFA0 = const.tile([128, S], BF16, tag="sfa0")
        nc.sync.dma_start(SFA0[:], sfa[0:128, :])
        SFA1 = const.tile([66, S], BF16, tag="sfa1")
        nc.sync.dma_start(SFA1[0:65, :], sfa[128:193, :])
        PT8 = []
        for j in range(8):
            t = const.tile([128, 2, CF + 1], F8, name=f"pt8{j}", tag=f"pt8{j}")
            nc.sync.dma_start(
                t[:], p8t.ap().rearrange("j p (two n) -> j p two n", two=2)[j]
            )
            PT8.append(t)
        WSP0 = const.tile([128, D], BF16, tag="wsp0")
        nc.sync.dma_start(WSP0[:], wsp[0:128, :])
        WSP1 = const.tile([65, D], BF16, tag="wsp1")
        nc.sync.dma_start(WSP1[:], wsp[128:193, :])
        WTN = []
        for c in range(2):
            w = const.tile([128, D], BF16, name=f"wtn{c}", tag=f"wtn{c}")
            nc.sync.dma_start(w[:], wtn[c * 128 : (c + 1) * 128, :])
            WTN.append(w)
        BTT = const.tile([128, D], F32, tag="btt")
        nc.sync.dma_start(BTT[:], btt[:, :])

        IDB = const.tile([128, 128], BF16, tag="idb")
        make_identity(nc, IDB[:])
        ID8 = const.tile([128, 128], F8, tag="id8")
        make_identity(nc, ID8[:])
        warm = const.tile([128, 512], BF16, tag="warm")
        nc.gpsimd.memset(warm[:], 0.0)
        for _ in range(12):
            wp = ps.tile([128, 512], F32, name="wps", tag="ps")
            nc.tensor.matmul(wp[:], lhsT=IDB[:], rhs=warm[:], start=True, stop=True)

        # ---- persistent SBUF tensors ----
        TtT = [hpool.tile([128, S], BF16, name=f"ttt{d}", tag=f"ttt{d}") for d in range(2)]
        G0 = hpool.tile([128, S], BF16, tag="g0")
        G1 = hpool.tile([66, S], BF16, tag="g1")
        TN = [hpool.tile([128, 2, C + 1], F8, name=f"tn{j}", tag=f"tn{j}") for j in range(8)]
        U2 = [hpool.tile([128, C + 1], F32, name=f"u2{k}", tag=f"u2{k}") for k in range(16)]
        RT = hpool.tile([128, 16], F32, tag="rt")
        RS = hpool.tile([128, 16], F32, tag="rs")

        # ---- BL: logit bias row -> SFA1 partition 65 ----
        # BL[s] = sum_cf P~[cf,s] * g0[cf]; lands in SFA row 193 so the S
        # matmul's second k-tile (66 parts) applies it against G1's ones row.
        # DVE cannot shift partitions, so stage at partition 0 and DMA across.
        BLS = hpool.tile([1, S], BF16, tag="bls")
        for sl in range(4):
            pbl = po.tile([1, 512], F32, name="pbl", tag="po")
            nc.tensor.matmul(
                pbl[:], lhsT=G0C0[:], rhs=SFA0[:, sl * 512 : (sl + 1) * 512],
                start=True, stop=False,
            )
            nc.tensor.matmul(
                pbl[:], lhsT=G0C1[:], rhs=SFA1[0:65, sl * 512 : (sl + 1) * 512],
                start=False, stop=True,
            )
            nc.vector.tensor_copy(BLS[:, sl * 512 : (sl + 1) * 512], pbl[:])
        nc.sync.dma_start(SFA1[65:66, :], BLS[:])

        # ---- conv: TtT[d][co, time-layout]; layout = [even 0:1024, odd 1024:2048]
        # wct rows: 0=W1(x[m],even) 1=W3(x[m-1],even) 2=W2(x[m],odd) 3=W0(x[m+1],odd)
        taps = [((0, 1), (1, 0)), ((2, 1), (3, 2))]  # (wct_idx, xt_offset)
        for d in range(2):
            for half in range(2):
                for ms in range(2):
                    p = ps.tile([128, 512], F32, name="cps", tag="ps")
                    n = 0
                    for ti, off in taps[half]:
                        for c in range(2):
                            nc.tensor.matmul(
                                p[:],
                                lhsT=WCT[ti][c][:, d * 128 : (d + 1) * 128],
                                rhs=XT[c][:, off + ms * 512 : off + ms * 512 + 512],
                                start=(n == 0),
                                stop=(n == 3),
                            )
                            n += 1
                    col = half * 1024 + ms * 512
                    if (half + ms) % 2 == 0:
                        nc.scalar.copy(TtT[d][:, col : col + 512], p[:])
                    else:
                        nc.vector.tensor_copy(TtT[d][:, col : col + 512], p[:])

        # ---- G = M @ T^T (bf16), plus ones row for the BL term ----
        # (full-tile memset: row 65 keeps 1.0, rows 0..64 overwritten below)
        nc.gpsimd.memset(G1[:], 1.0)
        for pg in range(2):
            for tsl in range(4):
                rows = 128 if pg == 0 else 65
                p = ps.tile([rows, 512], F32, name="gps", tag="ps")
                for c in range(2):
                    nc.tensor.matmul(
                        p[:],
                        lhsT=MT[c][:, pg * 128 : pg * 128 + rows],
                        rhs=TtT[c][:, tsl * 512 : (tsl + 1) * 512],
                        start=(c == 0),
                        stop=(c == 1),
                    )
                dst = G0 if pg == 0 else G1[0:65, :]
                if (pg + tsl) % 2 == 0:
                    nc.vector.tensor_copy(dst[:, tsl * 512 : (tsl + 1) * 512], p[:])
                else:
                    nc.scalar.copy(dst[:, tsl * 512 : (tsl + 1) * 512], p[:])

        # ---- TN: T natural-layout fp8 pairs [time-part, pair, 257] ----
        for j in range(8):
            nc.gpsimd.memset(TN[j][:, :, C : C + 1], 1.0)
            tp = ptr.tile([128, 2, 256], BF16, name="tnp", tag="ptr")
            for i in range(2):
                for cc in range(2):
                    nc.tensor.transpose(
                        tp[:, i, cc * 128 : (cc + 1) * 128],
                        TtT[cc][:, (2 * j + i) * 128 : (2 * j + i + 1) * 128],
                        IDB[:],
                    )
            nc.vector.tensor_copy(TN[j][:, :, 0:C], tp[:])

        # ---- attention ----
        for tsl in range(4):
            EP = [None] * 8
            for sc in range(16):
                j, i = sc // 2, sc % 2
                # S tile [spec 128, time 512] inc. logit bias via row 193
                p = ps.tile([128, 512], F32, name="sps", tag="ps")
                nc.tensor.matmul(
                    p[:], lhsT=SFA0[:, sc * 128 : (sc + 1) * 128],
                    rhs=G0[:, tsl * 512 : (tsl + 1) * 512], start=True, stop=False,
                )
                nc.tensor.matmul(
                    p[:], lhsT=SFA1[:, sc * 128 : (sc + 1) * 128],
                    rhs=G1[:, tsl * 512 : (tsl + 1) * 512], start=False, stop=True,
                )
                if i == 0:
                    EP[j] = epool.tile([128, 2, 512], F8, name="ep", tag="ep")
                EPj = EP[j]
                nc.scalar.activation(EPj[:, i, :], p[:], EXP, scale=SCALE)

                # eTs: fp8 transposes (element step 2) -> compact fp8
                etp = pe8.tile([128, 4, 128, 2], F8, name="etp", tag="pe8")
                for q in range(4):
                    nc.tensor.transpose(
                        etp[:, q, :, 0], EPj[:, i, q * 128 : (q + 1) * 128], ID8[:]
                    )
                eTs = tpool.tile([128, 4, 128], F8, name="ets", tag="ets")
                if sc % 2 == 0:
                    nc.vector.tensor_copy(eTs[:], etp[:, :, :, 0])
                else:
                    nc.scalar.copy(eTs[:], etp[:, :, :, 0])

                # U: fp8 DR over time pairs within this tsl slice
                up = pu.tile([128, C + 1], F32, name="up", tag="pu")
                for u in range(2):
                    nc.tensor.matmul(
                        up[:],
                        lhsT=eTs[:, 2 * u : 2 * u + 2, :],
                        rhs=TN[tsl * 2 + u][:],
                        start=(u == 0),
                        stop=(u == 1),
                        perf_mode=DR,
                    )
                if tsl == 0:
                    nc.vector.tensor_copy(U2[sc][:], up[:])
                else:
                    nc.vector.tensor_tensor(U2[sc][:], U2[sc][:], up[:], op=ADD)

                # fused_spec finalize (last tsl): normalize, bias, project
                if tsl == 3:
                    nc.vector.reciprocal(RS[:, sc : sc + 1], U2[sc][:, C : C + 1])
                    usb = spool.tile([128, C], BF16, name="usb", tag="usb")
                    nc.vector.tensor_copy(usb[:], U2[sc][:, 0:C])
                    trp = ptr.tile([128, 2, 256], BF16, name="utp", tag="ptr")
                    for cc in range(2):
                        nc.tensor.transpose(
                            trp[:, cc, 0:128], usb[:, cc * 128 : (cc + 1) * 128], IDB[:]
                        )
                    ust = spool.tile([128, 2, 128], BF16, name="ust", tag="ust")
                    nc.vector.tensor_copy(ust[:], trp[:, :, 0:128])
                    os_ = po.tile([128, D], F32, name="osp", tag="po")
                    for cc in range(2):
                        nc.tensor.matmul(
                            os_[:], lhsT=ust[:, cc, :], rhs=WTN[cc][:],
                            start=(cc == 0), stop=(cc == 1),
                        )
                    o2 = spool.tile([128, D], F32, name="o2", tag="o")
                    nc.scalar.mul(o2[:], os_[:], RS[:, sc : sc + 1])
                    nc.vector.tensor_tensor(o2[:], o2[:], BTT[:], op=ADD)
                    nc.sync.dma_start(out[sc * 128 : (sc + 1) * 128, D : 2 * D], o2[:])

            # R + fused_time finalize for this tsl (separate PSUM accumulation
            # groups: interleaved groups within one bank corrupt each other)
            for q in range(4):
                k = tsl * 4 + q
                RP = pr.tile([128, CF + 1], F32, name="rp", tag="rp", bufs=2)
                for j in range(8):
                    nc.tensor.matmul(
                        RP[:],
                        lhsT=EP[j][:, :, q * 128 : (q + 1) * 128],
                        rhs=PT8[j][:],
                        start=(j == 0),
                        stop=(j == 7),
                        perf_mode=DR,
                    )
                rp = RP[:]
                nc.vector.reciprocal(RT[:, k : k + 1], rp[:, CF : CF + 1])
                rsb = spool.tile([128, CF + 1], BF16, name="rsb", tag="rsb")
                nc.vector.tensor_copy(rsb[:], rp[:])
                trp = ptr.tile([128, 2, 256], BF16, name="rtp", tag="ptr")
                nc.tensor.transpose(trp[:, 0, 0:128], rsb[:, 0:128], IDB[:])
                nc.tensor.transpose(trp[0:65, 1, 0:128], rsb[:, 128:193], IDB[:])
                rst0 = spool.tile([128, 128], BF16, name="rst0", tag="rst0")
                nc.vector.tensor_copy(rst0[:], trp[:, 0, 0:128])
                rst1 = spool.tile([65, 128], BF16, name="rst1", tag="rst1")
                nc.vector.tensor_copy(rst1[:], trp[0:65, 1, 0:128])
                ot = po.tile([128, D], F32, name="otp", tag="po")
                nc.tensor.matmul(ot[:], lhsT=rst0[:], rhs=WSP0[:], start=True, stop=False)
                nc.tensor.matmul(ot[:], lhsT=rst1[:], rhs=WSP1[:], start=False, stop=True)
                o1 = spool.tile([128, D], F32, name="o1", tag="o")
                nc.scalar.mul(o1[:], ot[:], RT[:, k : k + 1])
                par, m0 = (0, k * 128) if k < 8 else (1, (k - 8) * 128)
                nc.sync.dma_start(out_r[par, m0 : m0 + 128, 0:D], o1[:])

    nc.compile()
    return nc


def make_in_maps(
    time_features,
    spec_features,
    w_conv,
    b_conv,
    w_tproj,
    b_tproj,
    w_sproj,
    b_sproj,
):
    time_features = np.asarray(time_features, np.float32)
    spec_features = np.asarray(spec_features, np.float32)
    w_conv = np.asarray(w_conv, np.float32)
    b_conv = np.asarray(b_conv, np.float32)
    w_tproj = np.asarray(w_tproj, np.float32)
    b_tproj = np.asarray(b_tproj, np.float32)
    w_sproj = np.asarray(w_sproj, np.float32)
    b_sproj = np.asarray(b_sproj, np.float32)

    # conv taps (pre-projection), order [W1, W3, W2, W0]
    wct = np.stack(
        [w_conv[:, :, 1], w_conv[:, :, 3], w_conv[:, :, 2], w_conv[:, :, 0]]
    ).astype(NPBF16)
    wsp_aug = np.concatenate([w_sproj.T, b_sproj[None, :]], 0)  # [193, 512]
    bt = b_conv @ w_tproj.T + b_tproj  # [512]
    M = wsp_aug @ w_tproj  # [193, 256]
    mT = np.ascontiguousarray(M.T).astype(NPBF16)  # [256, 193]
    g0 = (wsp_aug @ bt).reshape(CF + 1, 1).astype(NPBF16)
    wsp = wsp_aug.astype(NPBF16)
    wtn = np.ascontiguousarray(w_tproj.T).astype(NPBF16)  # [256, 512]
    btt = np.broadcast_to(bt.astype(np.float32), (128, D)).copy()

    in_maps = []
    for b in range(B):
        xt = np.zeros((C, L + 2), NPBF16)
        xt[:, 1 : L + 1] = time_features[b].T.astype(NPBF16)
        P_aug = np.concatenate(
            [spec_features[b].reshape(CF, S), np.ones((1, S), np.float32)], 0
        )
        sfa = P_aug.astype(NPBF16)
        p8t = np.ascontiguousarray(
            P_aug.T.reshape(8, 2, 128, CF + 1).transpose(0, 2, 1, 3).reshape(
                8, 128, 2 * (CF + 1)
            )
        ).astype(NPF8)
        in_maps.append(
            {
                "xt": xt,
                "wct": wct,
                "mT": mT,
                "g0c": g0,
                "sfa": sfa,
                "p8t": p8t,
                "wsp": wsp,
                "wtn": wtn,
                "btt": btt,
            }
        )
    return in_maps


_NC_CACHE = None


def get_nc():
    global _NC_CACHE
    if _NC_CACHE is None:
        _NC_CACHE = build_nc()
    return _NC_CACHE


def kernel(**inputs) -> np.ndarray:
    nc = get_nc()
    in_maps = make_in_maps(**inputs)
    res = run_bass_kernel_spmd(nc, in_maps, list(range(B)))
    return np.stack([res.results[i]["out"] for i in range(B)])


if __name__ == "__main__":
    rng = np.random.default_rng(0)
    ins = {
        "time_features": rng.standard_normal((B, L, C)).astype(np.float32),
        "spec_features": rng.standard_normal((B, 3, 64, S)).astype(np.float32),
        "w_conv": (rng.standard_normal((C, C, 4)) * 0.05).astype(np.float32),
        "b_conv": (rng.standard_normal(C) * 0.05).astype(np.float32),
        "w_tproj": (rng.standard_normal((D, C)) * 0.05).astype(np.float32),
        "b_tproj": (rng.standard_normal(D) * 0.05).astype(np.float32),
        "w_sproj": (rng.standard_normal((D, CF)) * 0.05).astype(np.float32),
        "b_sproj": (rng.standard_normal(D) * 0.05).astype(np.float32),
    }
    out = kernel(**ins)
    print("out", out.shape, out.dtype, float(np.abs(out).max()))
